# revision 53
# baseline (speedup 1.0000x reference)
"""Bayesian linear layer (reparameterized sample + predictive uncertainty)
as an 8-core SPMD Trainium2 Bass kernel.

Reference computation (all fp32):
    W     = weight_mu + exp(weight_log_sigma) * eps_w          # [OUT, IN]
    b     = bias_mu + exp(bias_log_sigma) * eps_b              # [OUT]
    out   = x @ W.T + b                                        # [B, OUT]
    unc   = sqrt((x*x) @ (exp(weight_log_sigma)**2).T + exp(bias_log_sigma)**2)

Sharding: 2 batch-halves x 4 out-feature-quarters = 8 cores.

Fast path (both log_sigma tensors constant, true for this module's inputs):
sigma/sigma_b are compile-time scalars, so
  unc[b, o] = sqrt(sigma^2 * rowsum(x[b]^2) + sigma_b^2)
is constant across o — the device computes one uncertainty value per batch
row and the host broadcasts it. The matmul runs in bf16 (tolerance 2e-2;
bf16 keeps absmax error ~3e-3): the host pre-transposes and pre-casts x /
weight_mu / eps_w into the exact K-major SBUF layouts the PE wants, so the
device does ZERO transposes and ZERO dtype-cast passes — just LDW+MM
streams, the reparameterization sample (DVE), the x^2 row-sums
(ACT/DVE + a ones-matmul partition reduction), and PSUM evacuation with
fused bias add (ACT Identity+bias).

General path (any log_sigma): the original f32 on-device-transpose kernel,
kept verbatim as a correctness fallback.
"""

import numpy as np

B, IN, OUT = 4096, 2048, 2048
R, C = 2, 4              # batch split x out-feature split
N_CORES = R * C
BS = B // R              # 2048 rows of x per core
OS = OUT // C            # 512 out features per core
KT = IN // 128           # 16 contraction k-tiles
BT = BS // 128           # 16 batch tiles per core (general path)
JT = OS // 128           # 4 out-feature partition-tiles per core
NBC = BS // 512          # 4 batch chunks of 512 (fast path)

# fast path batch phases: small first phase so the DMA-paced opening
# (which must wait for all 4MB of weights) carries little x alongside,
# small last phase to shrink the kernel tail.
PH_W = [256, 512, 512, 512, 256]        # columns per phase
PH_OFF = [0, 256, 768, 1280, 1792]      # batch offset of each phase
PH_FOFF = [16 * o for o in PH_OFF]      # free offset of phase block in xT

TRACE = False            # test harness sets True to capture an NTFF profile
LAST_RESULT = None       # BassKernelResults of the most recent run

_compiled = {}           # cache: key -> compiled Bass program


# ---------------------------------------------------------------------------
# fast path: constant weight_log_sigma AND constant bias_log_sigma
# ---------------------------------------------------------------------------

def _build_fast(sigma, sigma_b):
    import concourse.mybir as mybir
    import concourse.tile as tile
    from concourse import bacc

    F32 = mybir.dt.float32
    BF16 = mybir.dt.bfloat16
    AF = mybir.ActivationFunctionType
    ALU = mybir.AluOpType

    sig2 = float(sigma) * float(sigma)
    sigb2 = float(sigma_b) * float(sigma_b)

    nc = bacc.Bacc("TRN2", target_bir_lowering=False, debug=False,
                   num_devices=N_CORES)

    # host-packed layouts (see kernel() for the packing):
    #   xT :  [128, 32768] free = bc*8192 + kt*512 + b_in   (K on partitions)
    #   weps: [128, 16384] free = kt*1024 + (mu: 0-511 | sigma*eps: 512-1023),
    #         each [128, 512] half free = jt*128 + p_out
    #   bv :  [8, 132] f32: cols 0-127: rows 0-3 bias_mu, 4-7 eps_b
    #         (o = row*128 + col); cols 128-131: the bias-sample selection
    #         matrix S (S[n,n]=1, S[4+n,n]=sigma_b) so that
    #         bcol = bv[:, :128].T @ S in one K=8 matmul.
    xT_d = nc.dram_tensor("xT_sh", [128, KT * BS], BF16,
                          kind="ExternalInput").ap()
    weps_d = nc.dram_tensor("weps_sh", [128, 2 * KT * OS], BF16,
                            kind="ExternalInput").ap()
    bv_d = nc.dram_tensor("bv_sh", [8, 132], F32, kind="ExternalInput").ap()
    o_d = nc.dram_tensor("o_sh", [OS, BS], BF16, kind="ExternalOutput").ap()
    u_d = nc.dram_tensor("u_sh", [1, BS], F32, kind="ExternalOutput").ap()

    with tile.TileContext(nc) as tc:
        with (
            tc.tile_pool(name="const", bufs=1) as cpool,
            tc.tile_pool(name="se", bufs=2) as sepool,
            tc.tile_pool(name="sq", bufs=2) as sqpool,
            tc.tile_pool(name="tree", bufs=2) as trpool,
            tc.tile_pool(name="outs", bufs=4) as opool,
            tc.tile_pool(name="pmain", bufs=8, space="PSUM") as ppool,
        ):
            xT = cpool.tile([128, KT * BS], BF16, name="xT")
            weps = cpool.tile([128, 2 * KT * OS], BF16, name="weps")
            w = cpool.tile([128, KT * OS], BF16, name="w")
            acc = [cpool.tile([128, 512], BF16, name=f"acc{p}")
                   for p in range(len(PH_W))]
            u_sb = cpool.tile([1, BS], F32, name="u_sb")
            ones_col = cpool.tile([128, 1], BF16, name="ones_col")
            bv = cpool.tile([8, 132], F32, name="bv")
            bcol = cpool.tile([128, 4], F32, name="bcol")
            sigb2_t = cpool.tile([1, 1], F32, name="sigb2_t")
            zcol = cpool.tile([128, 1], F32, name="zcol")
            dum = cpool.tile([128, 512], BF16, name="dum")

            # --- input DMA stream ---------------------------------------
            # One HWDGE ring (sync) drains its queue FIFO, so arrival
            # order == issue order. Weight (mu|sigma*eps interleaved
            # host-side per kt) and bc0's x arrive strictly in PE
            # consumption order with graduated piece sizes — tiny pieces
            # first so the stream can start ~1us after data flows, big
            # pieces later to keep the issue count low. The bias vector
            # is [8, 132] (8 fat descriptors) so it can't throttle the
            # packet round-robin like a [128, 8] load would.
            nc.sync.dma_start(bv[:], bv_d[:])
            W_PIECES = [(0, 1), (1, 2), (2, 4), (4, 8), (8, 12), (12, 16)]
            w0 = PH_W[0]
            for a, b in W_PIECES:
                nc.sync.dma_start(weps[:, a * 1024:b * 1024],
                                  weps_d[:, a * 1024:b * 1024])
                nc.sync.dma_start(xT[:, a * w0:b * w0],
                                  xT_d[:, a * w0:b * w0])
            for h in range(2, 16):  # phases 1-4 in 512KB pieces
                sl = slice(h * 2048, (h + 1) * 2048)
                nc.sync.dma_start(xT[:, sl], xT_d[:, sl])

            # --- small prep on DVE --------------------------------------
            nc.vector.memset(dum[:], 1.0)
            nc.vector.memset(ones_col[:], 1.0)
            nc.vector.memset(sigb2_t[:], sigb2)
            nc.vector.memset(zcol[:], 0.0)

            # --- PE warm-up: matmuls on dummy data during the DMA ramp
            # flip the HAM clock-gate to 8/8 before the real stream and
            # bridge the idle window until the first weights land. One
            # accumulation chain so the framework can't elide them.
            warm = ppool.tile([1, 512], F32, tag="po", name="warm")
            NWARM = 8
            for i in range(NWARM):
                nc.tensor.matmul(warm[:], ones_col[:], dum[:],
                                 start=(i == 0), stop=(i == NWARM - 1))

            # bias column via PE: bcol[p, n] = sum_k bv[k, p] * S[k, n]
            pb = ppool.tile([128, 4], F32, tag="po", name="pb")
            nc.tensor.matmul(pb[:], bv[:, 0:128], bv[:, 128:132],
                             start=True, stop=True)
            nc.vector.tensor_copy(bcol[:], pb[:])

            # --- weight sample w = mu + (sigma*eps) (DVE, per kt); the
            # host pre-scales eps by sigma during the bf16 cast, so the
            # reparameterization combine is a single add.
            for kt in range(KT):
                nc.vector.tensor_tensor(
                    w[:, kt * 512:(kt + 1) * 512],
                    weps[:, kt * 1024:kt * 1024 + 512],
                    weps[:, kt * 1024 + 512:(kt + 1) * 1024],
                    ALU.add)



            # --- x^2 row-sum partials for one phase ---------------------
            # squares: first half on ACT, second on DVE; kt halving-tree
            # on DVE; result acc[p] [:, :w] bf16 (sum over kt of xT^2).
            def emit_usq(p):
                w_, f0 = PH_W[p], PH_FOFF[p]
                n = 16 * w_
                blk = xT[:, f0:f0 + n]
                xsq = sqpool.tile([128, 8192], BF16, tag="xsq",
                                  name=f"xsq{p}")
                nc.scalar.activation(xsq[:, :n // 2], blk[:, :n // 2],
                                     AF.Square, bias=zcol[:])
                nc.vector.tensor_tensor(xsq[:, n // 2:n], blk[:, n // 2:],
                                        blk[:, n // 2:], ALU.mult)
                t1 = trpool.tile([128, 4096], BF16, tag="t1", name=f"t1_{p}")
                nc.vector.tensor_tensor(t1[:, :n // 2], xsq[:, :n // 2],
                                        xsq[:, n // 2:n], ALU.add)
                t2 = trpool.tile([128, 2048], BF16, tag="t2", name=f"t2_{p}")
                nc.vector.tensor_tensor(t2[:, :n // 4], t1[:, :n // 4],
                                        t1[:, n // 4:n // 2], ALU.add)
                t3 = trpool.tile([128, 1024], BF16, tag="t3", name=f"t3_{p}")
                nc.vector.tensor_tensor(t3[:, :n // 8], t2[:, :n // 8],
                                        t2[:, n // 8:n // 4], ALU.add)
                nc.vector.tensor_tensor(acc[p][:, :w_], t3[:, :w_],
                                        t3[:, w_:n // 8], ALU.add)

            # --- uncertainty: partition-reduce acc[p] via ones-matmul ---
            def emit_pu_mm(p):
                w_ = PH_W[p]
                pu = ppool.tile([1, 512], F32, tag="po", name=f"pu{p}")
                nc.tensor.matmul(pu[:, :w_], ones_col[:], acc[p][:, :w_],
                                 start=True, stop=True)
                return pu

            def emit_pu_sqrt(p, pu):
                w_, o0 = PH_W[p], PH_OFF[p]
                nc.scalar.activation(u_sb[:, o0:o0 + w_], pu[:, :w_],
                                     AF.Sqrt, scale=sig2, bias=sigb2_t[:])

            def emit_mm(po, p, jt, kt):
                w_, f0 = PH_W[p], PH_FOFF[p]
                nc.tensor.matmul(
                    po[:, :w_],
                    w[:, kt * OS + jt * 128:kt * OS + (jt + 1) * 128],
                    xT[:, f0 + kt * w_:f0 + (kt + 1) * w_],
                    start=(kt == 0), stop=(kt == KT - 1))

            def emit_evac(po, p, jt):
                w_, o0 = PH_W[p], PH_OFF[p]
                o_t = opool.tile([128, 512], BF16, tag="o",
                                 name=f"o{p}_{jt}")
                if jt % 2 == 0:
                    nc.scalar.activation(o_t[:, :w_], po[:, :w_],
                                         AF.Identity,
                                         bias=bcol[:, jt:jt + 1])
                else:
                    nc.vector.tensor_scalar(o_t[:, :w_], po[:, :w_],
                                            bcol[:, jt:jt + 1], None,
                                            ALU.add)
                eng = nc.sync if jt % 2 == 0 else nc.scalar
                eng.dma_start(o_d[jt * 128:(jt + 1) * 128, o0:o0 + w_],
                              o_t[:, :w_])

            # --- main matmul: out.T[o, b] = sum_k W[o, k] x[b, k] -------
            # phase 0 (256 cols): kt outer, jt mid — consumes weight
            # pieces in DMA arrival order with minimal x alongside.
            # phases 1-4 (data resident by then): jt-outer so chains
            # complete staggered and evac/store pipeline off the tail.
            # The pu (uncertainty) matmuls + sqrts are slipped into the
            # middle of the stream so nothing u-related trails.
            pos = [ppool.tile([128, 512], F32, tag="po",
                              name=f"po0_{jt}") for jt in range(JT)]
            emit_usq(0)
            for kt in range(KT):
                for jt in range(JT):
                    emit_mm(pos[jt], 0, jt, kt)
            for jt in range(JT):
                emit_evac(pos[jt], 0, jt)

            NP = len(PH_W)
            for p in range(1, NP):
                emit_usq(p)
                for jt in range(JT):
                    po = ppool.tile([128, 512], F32, tag="po",
                                    name=f"po{p}_{jt}")
                    for kt in range(KT):
                        emit_mm(po, p, jt, kt)
                    emit_evac(po, p, jt)
                    # mid-stream u work on finished acc's
                    if jt == 1:
                        pu = emit_pu_mm(p - 1)
                        emit_pu_sqrt(p - 1, pu)
                    if jt == 2 and p == NP - 1:
                        pu = emit_pu_mm(p)
                        emit_pu_sqrt(p, pu)
                        nc.scalar.dma_start(u_d[:], u_sb[:])

    nc.compile()
    return nc


def _pack_xT(x_half_bf):
    """[2048(b), 2048(k)] bf16 -> [128, 32768]: per-phase blocks, block p
    at free [16*PH_OFF[p] ...] with free = kt*PH_W[p] + b_in inside."""
    xk = x_half_bf.T  # [2048(k), 2048(b)]
    parts = []
    for o0, w_ in zip(PH_OFF, PH_W):
        parts.append(xk[:, o0:o0 + w_].reshape(KT, 128, w_)
                     .transpose(1, 0, 2).reshape(128, KT * w_))
    return np.ascontiguousarray(np.concatenate(parts, axis=1))


def _pack_wT(wq_bf):
    """[512(o), 2048(k)] bf16 -> [128, 8192] free = kt*512 + o."""
    return np.ascontiguousarray(
        wq_bf.T.reshape(KT, 128, OS).transpose(1, 0, 2).reshape(128, KT * OS))


def _pack_weps(muq_bf, epsq_bf):
    """Interleave packed mu/eps per kt: [128, 16384] where kt spans
    free [kt*1024, (kt+1)*1024) = [mu kt-slice | eps kt-slice]."""
    muP = _pack_wT(muq_bf).reshape(128, KT, OS)
    epsP = _pack_wT(epsq_bf).reshape(128, KT, OS)
    return np.ascontiguousarray(
        np.stack([muP, epsP], axis=2).reshape(128, 2 * KT * OS))


# ---------------------------------------------------------------------------
# general path: original f32 kernel (on-device transposes), kept as fallback
# ---------------------------------------------------------------------------

def _build_general():
    import concourse.mybir as mybir
    import concourse.tile as tile
    from concourse import bacc
    from concourse.masks import make_identity

    F32 = mybir.dt.float32
    F32R = mybir.dt.float32r
    AF = mybir.ActivationFunctionType
    ALU = mybir.AluOpType

    nc = bacc.Bacc("TRN2", target_bir_lowering=False, debug=False,
                   num_devices=N_CORES)

    x_d = nc.dram_tensor("x_sh", [BS, IN], F32R, kind="ExternalInput").ap()
    mu_d = nc.dram_tensor("mu_sh", [OS, IN], F32, kind="ExternalInput").ap()
    eps_d = nc.dram_tensor("eps_sh", [OS, IN], F32, kind="ExternalInput").ap()
    ls_d = nc.dram_tensor("ls_sh", [OS, IN], F32, kind="ExternalInput").ap()
    bmu_d = nc.dram_tensor("bmu_sh", [1, OS], F32, kind="ExternalInput").ap()
    bls_d = nc.dram_tensor("bls_sh", [1, OS], F32, kind="ExternalInput").ap()
    beps_d = nc.dram_tensor("beps_sh", [1, OS], F32, kind="ExternalInput").ap()
    o_d = nc.dram_tensor("o_sh", [BS, OS], F32, kind="ExternalOutput").ap()
    u_d = nc.dram_tensor("u_sh", [BS, OS], F32, kind="ExternalOutput").ap()

    with tile.TileContext(nc) as tc:
        with (
            tc.tile_pool(name="const", bufs=1) as cpool,
            tc.tile_pool(name="wres", bufs=1) as wres,
            tc.tile_pool(name="psum", bufs=3, space="PSUM") as ppool,
        ):
            ident_f = cpool.tile([128, 128], F32)
            make_identity(nc, ident_f)
            ident = cpool.tile([128, 128], F32R)
            nc.vector.tensor_copy(ident[:], ident_f[:])
            ones_f = cpool.tile([1, 128], F32)
            nc.vector.memset(ones_f[:], 1.0)
            ones1 = cpool.tile([1, 128], F32R)
            nc.vector.tensor_copy(ones1[:], ones_f[:])

            # --- weight prep: WsampT and S2T as KT k-tiles [128, OS] f32r
            wT = [wres.tile([128, OS], F32R, tag=f"wT{i}", name=f"wT{i}")
                  for i in range(KT)]
            s2T = [wres.tile([128, OS], F32R, tag=f"s2T{i}", name=f"s2T{i}")
                   for i in range(KT)]

            with (
                tc.tile_pool(name="wprep", bufs=2) as wpool,
                tc.tile_pool(name="xs", bufs=3) as xpool,
                tc.tile_pool(name="outs", bufs=3) as opool,
                tc.tile_pool(name="po", bufs=2, space="PSUM") as popool,
            ):
                state = {}

                HI = IN // 2

                def emit_jt(jt, h):
                    sl = slice(jt * 128, (jt + 1) * 128)
                    fsl = slice(h * HI, (h + 1) * HI)
                    mu_t = wpool.tile([128, HI], F32, tag="mu", name="mu_t",
                                      bufs=4)
                    eps_t = wpool.tile([128, HI], F32, tag="eps", name="eps_t",
                                       bufs=4)
                    nc.sync.dma_start(mu_t[:], mu_d[sl, fsl])
                    nc.sync.dma_start(eps_t[:], eps_d[sl, fsl])
                    w_t = wpool.tile([128, HI], F32R, tag="w", name="w_t",
                                     bufs=2)
                    ls_t = wpool.tile([128, HI], F32, tag="ls", name="ls_t",
                                      bufs=3)
                    nc.sync.dma_start(ls_t[:], ls_d[sl, fsl])
                    sig_t = wpool.tile([128, HI], F32, tag="sig",
                                       name="sig_t", bufs=2)
                    nc.scalar.activation(sig_t[:], ls_t[:], AF.Exp)
                    se_t = wpool.tile([128, HI], F32, tag="se", bufs=2,
                                      name="se_t")
                    nc.vector.tensor_tensor(se_t[:], sig_t[:], eps_t[:],
                                            ALU.mult)
                    nc.vector.tensor_tensor(w_t[:], mu_t[:], se_t[:], ALU.add)
                    s2_t = wpool.tile([128, HI], F32R, tag="s2", name="s2_t",
                                      bufs=2)
                    nc.scalar.activation(s2_t[:], sig_t[:], AF.Square)

                    k0 = h * (KT // 2)
                    for src_t, dst in ((w_t, wT), (s2_t, s2T)):
                        for g in range(KT // 8):
                            pt = ppool.tile([128, 512], F32R, tag="tp",
                                            name="pt")
                            for ii in range(4):
                                i = 4 * g + ii
                                nc.tensor.transpose(
                                    pt[:, ii * 128:(ii + 1) * 128],
                                    src_t[:, i * 128:(i + 1) * 128], ident[:])
                            for ii in range(4):
                                i = 4 * g + ii
                                nc.any.tensor_copy(
                                    dst[k0 + i][:, jt * 128:(jt + 1) * 128],
                                    pt[:, ii * 128:(ii + 1) * 128])

                def emit_front(bt):
                    x_t = xpool.tile([128, IN], F32R, tag="x", bufs=2,
                                     name="x_t")
                    dma_eng = nc.sync if bt % 2 == 0 else nc.scalar
                    dma_eng.dma_start(x_t[:], x_d[bt * 128:(bt + 1) * 128, :])
                    xT = xpool.tile([128, KT * 128], F32R, tag="xT", bufs=3,
                                    name="xT")
                    for g in range(KT // 4):
                        pt = ppool.tile([128, 512], F32R, tag="tp", name="pt")
                        for ii in range(4):
                            i = 4 * g + ii
                            nc.tensor.transpose(
                                pt[:, ii * 128:(ii + 1) * 128],
                                x_t[:, i * 128:(i + 1) * 128], ident[:])
                        nc.any.tensor_copy(xT[:, g * 512:(g + 1) * 512], pt[:])
                    state[bt] = xT

                def emit_back(bt):
                    xT = state.pop(bt)
                    po = popool.tile([128, OS], F32, tag="po", name="po")
                    for i in range(KT):
                        nc.tensor.matmul(po[:], xT[:, i * 128:(i + 1) * 128],
                                         wT[i][:], start=(i == 0),
                                         stop=(i == KT - 1))
                    o_t = opool.tile([128, OS], F32, tag="o", name="o_t",
                                     bufs=2)
                    nc.vector.tensor_tensor(o_t[:], po[:], bias_bc[:], ALU.add)
                    nc.sync.dma_start(o_d[bt * 128:(bt + 1) * 128, :], o_t[:])

                    u_t = opool.tile([128, OS], F32, tag="u", name="u_t",
                                     bufs=2)
                    x2T = xpool.tile([128, KT * 128], F32R, tag="x2T",
                                     bufs=1, name="x2T")
                    nc.scalar.activation(x2T[:], xT[:].bitcast(F32),
                                         AF.Square)
                    pu = popool.tile([128, OS], F32, tag="pu", name="pu",
                                     bufs=2)
                    for i in range(KT):
                        nc.tensor.matmul(pu[:],
                                         x2T[:, i * 128:(i + 1) * 128],
                                         s2T[i][:], start=(i == 0),
                                         stop=False)
                    nc.tensor.matmul(pu[:], ones1[:], bs2_r[:],
                                     start=False, stop=True)
                    nc.scalar.activation(u_t[:], pu[:], AF.Sqrt)
                    nc.sync.dma_start(u_d[bt * 128:(bt + 1) * 128, :], u_t[:])

                for jt in range(JT):
                    for h in range(2):
                        emit_jt(jt, h)

                # bias rows: b_samp = bmu + exp(bls)*beps ; bs2 = exp(2*bls)
                bmu_r = cpool.tile([1, OS], F32)
                bls_r = cpool.tile([1, OS], F32)
                beps_r = cpool.tile([1, OS], F32)
                nc.scalar.dma_start(bmu_r[:], bmu_d[:])
                nc.scalar.dma_start(bls_r[:], bls_d[:])
                nc.scalar.dma_start(beps_r[:], beps_d[:])
                bsig_r = cpool.tile([1, OS], F32)
                nc.scalar.activation(bsig_r[:], bls_r[:], AF.Exp)
                bse_r = cpool.tile([1, OS], F32)
                nc.vector.tensor_tensor(bse_r[:], bsig_r[:], beps_r[:],
                                        ALU.mult)
                bias_r = cpool.tile([1, OS], F32R)
                nc.vector.tensor_tensor(bias_r[:], bmu_r[:], bse_r[:], ALU.add)
                bs2_r = cpool.tile([1, OS], F32R)
                nc.vector.tensor_tensor(bs2_r[:], bsig_r[:], bsig_r[:],
                                        ALU.mult)

                # broadcast bias row across partitions (K=1 ones matmul)
                pb = ppool.tile([128, OS], F32, tag="tp")
                nc.tensor.matmul(pb[:], ones1[:], bias_r[:], start=True,
                                 stop=True)
                bias_bc = cpool.tile([128, OS], F32)
                nc.any.tensor_copy(bias_bc[:], pb[:])

                for bt in range(BT):
                    emit_front(bt)
                    emit_back(bt)

    nc.compile()
    return nc


# ---------------------------------------------------------------------------
# host wrapper
# ---------------------------------------------------------------------------

def kernel(x, weight_mu, weight_log_sigma, bias_mu, bias_log_sigma,
           eps_w, eps_b):
    global LAST_RESULT
    import ml_dtypes
    from concourse.bass_utils import run_bass_kernel_spmd

    BF = ml_dtypes.bfloat16

    x = np.ascontiguousarray(np.asarray(x, dtype=np.float32))
    weight_mu = np.asarray(weight_mu, dtype=np.float32)
    weight_log_sigma = np.asarray(weight_log_sigma, dtype=np.float32)
    bias_mu = np.asarray(bias_mu, dtype=np.float32).reshape(OUT)
    bias_log_sigma = np.asarray(bias_log_sigma, dtype=np.float32).reshape(OUT)
    eps_w = np.asarray(eps_w, dtype=np.float32)
    eps_b = np.asarray(eps_b, dtype=np.float32).reshape(OUT)

    ls0 = weight_log_sigma.flat[0]
    bls0 = bias_log_sigma.flat[0]
    fast = bool(np.all(weight_log_sigma == ls0)) and bool(
        np.all(bias_log_sigma == bls0))

    if fast:
        sigma = float(np.exp(np.float32(ls0)))
        sigma_b = float(np.exp(np.float32(bls0)))
        key = ("fast", sigma, sigma_b)
        if key not in _compiled:
            _compiled[key] = _build_fast(sigma, sigma_b)
        nc = _compiled[key]

        x_bf = x.astype(BF)
        xT_halves = [_pack_xT(x_bf[i * BS:(i + 1) * BS]) for i in range(R)]
        mu_bf = weight_mu.astype(BF)
        eps_bf = (eps_w * np.float32(sigma)).astype(BF)
        in_maps = []
        for i in range(R):
            for j in range(C):
                bv = np.zeros((8, 132), dtype=np.float32)
                bv[0:4, 0:128] = bias_mu[j * OS:(j + 1) * OS].reshape(4, 128)
                bv[4:8, 0:128] = eps_b[j * OS:(j + 1) * OS].reshape(4, 128)
                for n in range(4):
                    bv[n, 128 + n] = 1.0
                    bv[4 + n, 128 + n] = sigma_b
                in_maps.append({
                    "xT_sh": xT_halves[i],
                    "weps_sh": _pack_weps(mu_bf[j * OS:(j + 1) * OS],
                                          eps_bf[j * OS:(j + 1) * OS]),
                    "bv_sh": bv,
                })
        res = run_bass_kernel_spmd(nc, in_maps, core_ids=list(range(N_CORES)),
                                   trace=TRACE)
        LAST_RESULT = res

        output = np.empty((B, OUT), dtype=np.float32)
        uncertainty = np.empty((B, OUT), dtype=np.float32)
        for i in range(R):
            for j in range(C):
                c = i * C + j
                output[i * BS:(i + 1) * BS, j * OS:(j + 1) * OS] = (
                    res.results[c]["o_sh"].astype(np.float32).T)
            u_row = res.results[i * C]["u_sh"].reshape(BS)
            uncertainty[i * BS:(i + 1) * BS, :] = u_row[:, None]
        return output, uncertainty

    # ----- general fallback (original kernel) -----
    key = ("general",)
    if key not in _compiled:
        _compiled[key] = _build_general()
    nc = _compiled[key]

    bias_mu2 = bias_mu.reshape(1, OUT)
    bias_log_sigma2 = bias_log_sigma.reshape(1, OUT)
    eps_b2 = eps_b.reshape(1, OUT)
    in_maps = []
    for i in range(R):
        for j in range(C):
            m = {
                "x_sh": x[i * BS:(i + 1) * BS],
                "mu_sh": weight_mu[j * OS:(j + 1) * OS],
                "eps_sh": eps_w[j * OS:(j + 1) * OS],
                "ls_sh": weight_log_sigma[j * OS:(j + 1) * OS],
                "bmu_sh": bias_mu2[:, j * OS:(j + 1) * OS],
                "bls_sh": bias_log_sigma2[:, j * OS:(j + 1) * OS],
                "beps_sh": eps_b2[:, j * OS:(j + 1) * OS],
            }
            in_maps.append({k: np.ascontiguousarray(v) for k, v in m.items()})

    res = run_bass_kernel_spmd(nc, in_maps, core_ids=list(range(N_CORES)),
                               trace=TRACE)
    LAST_RESULT = res

    output = np.empty((B, OUT), dtype=np.float32)
    uncertainty = np.empty((B, OUT), dtype=np.float32)
    for i in range(R):
        for j in range(C):
            c = i * C + j
            output[i * BS:(i + 1) * BS,
                   j * OS:(j + 1) * OS] = res.results[c]["o_sh"]
            uncertainty[i * BS:(i + 1) * BS,
                        j * OS:(j + 1) * OS] = res.results[c]["u_sh"]
    return output, uncertainty


# revision 54
# speedup vs baseline: 1.0297x; 1.0297x over previous
"""Bayesian linear layer (reparameterized sample + predictive uncertainty)
as an 8-core SPMD Trainium2 Bass kernel.

Reference computation (all fp32):
    W     = weight_mu + exp(weight_log_sigma) * eps_w          # [OUT, IN]
    b     = bias_mu + exp(bias_log_sigma) * eps_b              # [OUT]
    out   = x @ W.T + b                                        # [B, OUT]
    unc   = sqrt((x*x) @ (exp(weight_log_sigma)**2).T + exp(bias_log_sigma)**2)

Sharding: 2 batch-halves x 4 out-feature-quarters = 8 cores.

Fast path (both log_sigma tensors constant, true for this module's inputs):
sigma/sigma_b are compile-time scalars, so
  unc[b, o] = sqrt(sigma^2 * rowsum(x[b]^2) + sigma_b^2)
is constant across o — the device computes one uncertainty value per batch
row and the host broadcasts it. The matmul runs in bf16 (tolerance 2e-2;
bf16 keeps absmax error ~3e-3): the host pre-transposes and pre-casts x /
weight_mu / eps_w into the exact K-major SBUF layouts the PE wants, so the
device does ZERO transposes and ZERO dtype-cast passes — just LDW+MM
streams, the reparameterization sample (DVE), the x^2 row-sums
(ACT/DVE + a ones-matmul partition reduction), and PSUM evacuation with
fused bias add (ACT Identity+bias).

General path (any log_sigma): the original f32 on-device-transpose kernel,
kept verbatim as a correctness fallback.
"""

import numpy as np

B, IN, OUT = 4096, 2048, 2048
R, C = 2, 4              # batch split x out-feature split
N_CORES = R * C
BS = B // R              # 2048 rows of x per core
OS = OUT // C            # 512 out features per core
KT = IN // 128           # 16 contraction k-tiles
BT = BS // 128           # 16 batch tiles per core (general path)
JT = OS // 128           # 4 out-feature partition-tiles per core
NBC = BS // 512          # 4 batch chunks of 512 (fast path)

TRACE = False            # test harness sets True to capture an NTFF profile
LAST_RESULT = None       # BassKernelResults of the most recent run

_compiled = {}           # cache: key -> compiled Bass program


# ---------------------------------------------------------------------------
# fast path: constant weight_log_sigma AND constant bias_log_sigma
# ---------------------------------------------------------------------------

def _build_fast(sigma, sigma_b):
    import concourse.mybir as mybir
    import concourse.tile as tile
    from concourse import bacc

    F32 = mybir.dt.float32
    BF16 = mybir.dt.bfloat16
    AF = mybir.ActivationFunctionType
    ALU = mybir.AluOpType

    sig2 = float(sigma) * float(sigma)
    sigb2 = float(sigma_b) * float(sigma_b)

    nc = bacc.Bacc("TRN2", target_bir_lowering=False, debug=False,
                   num_devices=N_CORES)

    # host-packed layouts (see kernel() for the packing):
    #   xT :  [128, 32768] free = bc*8192 + kt*512 + b_in   (K on partitions)
    #   weps: [128, 16384] free = kt*1024 + (mu: 0-511 | sigma*eps: 512-1023),
    #         each [128, 512] half free = jt*128 + p_out
    #   bv :  [8, 132] f32: cols 0-127: rows 0-3 bias_mu, 4-7 eps_b
    #         (o = row*128 + col); cols 128-131: the bias-sample selection
    #         matrix S (S[n,n]=1, S[4+n,n]=sigma_b) so that
    #         bcol = bv[:, :128].T @ S in one K=8 matmul.
    xT_d = nc.dram_tensor("xT_sh", [128, KT * BS], BF16,
                          kind="ExternalInput").ap()
    weps_d = nc.dram_tensor("weps_sh", [128, 2 * KT * OS], BF16,
                            kind="ExternalInput").ap()
    bv_d = nc.dram_tensor("bv_sh", [8, 132], F32, kind="ExternalInput").ap()
    o_d = nc.dram_tensor("o_sh", [OS, BS], BF16, kind="ExternalOutput").ap()
    u_d = nc.dram_tensor("u_sh", [1, BS], F32, kind="ExternalOutput").ap()

    with tile.TileContext(nc) as tc:
        with (
            tc.tile_pool(name="const", bufs=1) as cpool,
            tc.tile_pool(name="se", bufs=2) as sepool,
            tc.tile_pool(name="sq", bufs=4) as sqpool,
            tc.tile_pool(name="tree", bufs=2) as trpool,
            tc.tile_pool(name="outs", bufs=4) as opool,
            tc.tile_pool(name="pmain", bufs=8, space="PSUM") as ppool,
        ):
            xT = cpool.tile([128, KT * BS], BF16, name="xT")
            weps = cpool.tile([128, 2 * KT * OS], BF16, name="weps")
            w = cpool.tile([128, KT * OS], BF16, name="w")
            acc = [cpool.tile([128, 512], BF16, name=f"acc{bc}")
                   for bc in range(NBC)]
            u_sb = cpool.tile([1, BS], F32, name="u_sb")
            ones_col = cpool.tile([128, 1], BF16, name="ones_col")
            bv = cpool.tile([8, 132], F32, name="bv")
            bcol = cpool.tile([128, 4], F32, name="bcol")
            sigb2_t = cpool.tile([1, 1], F32, name="sigb2_t")
            zcol = cpool.tile([128, 1], F32, name="zcol")
            dum = cpool.tile([128, 512], BF16, name="dum")

            # --- input DMA stream ---------------------------------------
            # One HWDGE ring (sync) drains its queue FIFO, so arrival
            # order == issue order. Weight (mu|sigma*eps interleaved
            # host-side per kt) and bc0's x arrive strictly in PE
            # consumption order with graduated piece sizes — tiny pieces
            # first so the stream can start ~1us after data flows, big
            # pieces later to keep the issue count low. The bias vector
            # is [8, 132] (8 fat descriptors) so it can't throttle the
            # packet round-robin like a [128, 8] load would.
            nc.sync.dma_start(bv[:], bv_d[:])
            W_PIECES = [(0, 1), (1, 2), (2, 4), (4, 8), (8, 12), (12, 16)]
            for a, b in W_PIECES:
                nc.sync.dma_start(weps[:, a * 1024:b * 1024],
                                  weps_d[:, a * 1024:b * 1024])
                nc.sync.dma_start(xT[:, a * 512:b * 512],
                                  xT_d[:, a * 512:b * 512])
            for h in range(2, 8):  # bc1-3 in 1MB halves
                sl = slice(h * 4096, (h + 1) * 4096)
                nc.sync.dma_start(xT[:, sl], xT_d[:, sl])

            # --- small prep on DVE --------------------------------------
            nc.vector.memset(dum[:], 1.0)
            nc.vector.memset(ones_col[:], 1.0)
            nc.vector.memset(sigb2_t[:], sigb2)
            nc.vector.memset(zcol[:], 0.0)

            # --- PE warm-up: matmuls on dummy data during the DMA ramp
            # flip the HAM clock-gate to 8/8 before the real stream and
            # bridge the idle window until the first weights land. One
            # accumulation chain so the framework can't elide them.
            warm = ppool.tile([1, 512], F32, tag="po", name="warm")
            NWARM = 8
            for i in range(NWARM):
                nc.tensor.matmul(warm[:], ones_col[:], dum[:],
                                 start=(i == 0), stop=(i == NWARM - 1))

            # bias column via PE: bcol[p, n] = sum_k bv[k, p] * S[k, n]
            pb = ppool.tile([128, 4], F32, tag="po", name="pb")
            nc.tensor.matmul(pb[:], bv[:, 0:128], bv[:, 128:132],
                             start=True, stop=True)
            nc.vector.tensor_copy(bcol[:], pb[:])

            # --- weight sample w = mu + (sigma*eps) (DVE, per kt); the
            # host pre-scales eps by sigma during the bf16 cast, so the
            # reparameterization combine is a single add.
            for kt in range(KT):
                nc.vector.tensor_tensor(
                    w[:, kt * 512:(kt + 1) * 512],
                    weps[:, kt * 1024:kt * 1024 + 512],
                    weps[:, kt * 1024 + 512:(kt + 1) * 1024],
                    ALU.add)



            # --- x^2 row-sum partials for one bc ------------------------
            # squares: quads 0-1 on ACT, 2-3 on DVE; halving-tree on DVE;
            # result acc[bc] [128, 512] bf16 (sum over kt of xT^2).
            def emit_usq(bc):
                qs = []
                for q in range(4):
                    src = xT[:, (bc * 4 + q) * 2048:(bc * 4 + q + 1) * 2048]
                    xsq = sqpool.tile([128, 2048], BF16, tag="xsq",
                                      name=f"xsq{bc}_{q}")
                    if q < 2:
                        nc.scalar.activation(xsq[:], src, AF.Square,
                                             bias=zcol[:])
                    else:
                        nc.vector.tensor_tensor(xsq[:], src, src, ALU.mult)
                    t1 = trpool.tile([128, 1024], BF16, tag="t1",
                                     name=f"t1_{bc}_{q}")
                    nc.vector.tensor_tensor(t1[:], xsq[:, :1024],
                                            xsq[:, 1024:], ALU.add)
                    t2 = trpool.tile([128, 512], BF16, tag="t2", bufs=4,
                                     name=f"t2_{bc}_{q}")
                    nc.vector.tensor_tensor(t2[:], t1[:, :512], t1[:, 512:],
                                            ALU.add)
                    qs.append(t2)
                ta = trpool.tile([128, 512], BF16, tag="t3",
                                 name=f"t3a_{bc}")
                nc.vector.tensor_tensor(ta[:], qs[0][:], qs[1][:], ALU.add)
                tb = trpool.tile([128, 512], BF16, tag="t3",
                                 name=f"t3b_{bc}")
                nc.vector.tensor_tensor(tb[:], qs[2][:], qs[3][:], ALU.add)
                nc.vector.tensor_tensor(acc[bc][:], ta[:], tb[:], ALU.add)

            # --- uncertainty: partition-reduce acc[bc] via ones-matmul --
            def emit_pu_mm(bc):
                pu = ppool.tile([1, 512], F32, tag="po", name=f"pu{bc}")
                nc.tensor.matmul(pu[:], ones_col[:], acc[bc][:],
                                 start=True, stop=True)
                return pu

            def emit_pu_sqrt(bc, pu):
                nc.scalar.activation(u_sb[:, bc * 512:(bc + 1) * 512], pu[:],
                                     AF.Sqrt, scale=sig2, bias=sigb2_t[:])

            def emit_mm(po, bc, jt, kt):
                nc.tensor.matmul(
                    po[:],
                    w[:, kt * OS + jt * 128:kt * OS + (jt + 1) * 128],
                    xT[:, bc * 8192 + kt * 512:bc * 8192 + (kt + 1) * 512],
                    start=(kt == 0), stop=(kt == KT - 1))

            def emit_evac(po, bc, jt):
                o_t = opool.tile([128, 512], BF16, tag="o",
                                 name=f"o{bc}_{jt}")
                if jt % 2 == 0:
                    nc.scalar.activation(o_t[:], po[:], AF.Identity,
                                         bias=bcol[:, jt:jt + 1])
                else:
                    nc.vector.tensor_scalar(o_t[:], po[:],
                                            bcol[:, jt:jt + 1], None,
                                            ALU.add)
                eng = nc.sync if jt % 2 == 0 else nc.scalar
                eng.dma_start(o_d[jt * 128:(jt + 1) * 128,
                                  bc * 512:(bc + 1) * 512], o_t[:])

            # --- main matmul: out.T[o, b] = sum_k W[o, k] x[b, k] -------
            # bc0: kt-pair outer, jt mid — consumes weight pieces in DMA
            # arrival order (psum held across each whole kt chain).
            # bc1-3 (data fully resident by then): jt-outer so the four
            # chains complete staggered and evac/store pipeline off the
            # tail. The pu (uncertainty) matmuls + sqrts are slipped into
            # the middle of the stream so nothing u-related trails.
            pos = [ppool.tile([128, 512], F32, tag="po",
                              name=f"po0_{jt}") for jt in range(JT)]
            emit_usq(0)
            for kt in range(KT):
                for jt in range(JT):
                    emit_mm(pos[jt], 0, jt, kt)
            for jt in range(JT):
                emit_evac(pos[jt], 0, jt)

            for bc in range(1, NBC):
                emit_usq(bc)
                for jt in range(JT):
                    po = ppool.tile([128, 512], F32, tag="po",
                                    name=f"po{bc}_{jt}")
                    for kt in range(KT):
                        emit_mm(po, bc, jt, kt)
                    emit_evac(po, bc, jt)
                    # mid-stream u work on finished acc's
                    if jt == 1 and bc >= 1:
                        pu = emit_pu_mm(bc - 1)
                        emit_pu_sqrt(bc - 1, pu)
                    if jt == 2 and bc == NBC - 1:
                        pu = emit_pu_mm(bc)
                        emit_pu_sqrt(bc, pu)
                        nc.scalar.dma_start(u_d[:], u_sb[:])

    nc.compile()
    return nc


def _pack_xT(x_half_bf):
    """[2048(b), 2048(k)] bf16 -> [128, 32768] free = bc*8192 + kt*512 + b_in."""
    return np.ascontiguousarray(
        x_half_bf.T.reshape(KT, 128, NBC, 512)
        .transpose(1, 2, 0, 3).reshape(128, KT * BS))


def _pack_wT(wq_bf):
    """[512(o), 2048(k)] bf16 -> [128, 8192] free = kt*512 + o."""
    return np.ascontiguousarray(
        wq_bf.T.reshape(KT, 128, OS).transpose(1, 0, 2).reshape(128, KT * OS))


def _pack_weps(muq_bf, epsq_bf):
    """Interleave packed mu/eps per kt: [128, 16384] where kt spans
    free [kt*1024, (kt+1)*1024) = [mu kt-slice | eps kt-slice]."""
    muP = _pack_wT(muq_bf).reshape(128, KT, OS)
    epsP = _pack_wT(epsq_bf).reshape(128, KT, OS)
    return np.ascontiguousarray(
        np.stack([muP, epsP], axis=2).reshape(128, 2 * KT * OS))


# ---------------------------------------------------------------------------
# general path: original f32 kernel (on-device transposes), kept as fallback
# ---------------------------------------------------------------------------

def _build_general():
    import concourse.mybir as mybir
    import concourse.tile as tile
    from concourse import bacc
    from concourse.masks import make_identity

    F32 = mybir.dt.float32
    F32R = mybir.dt.float32r
    AF = mybir.ActivationFunctionType
    ALU = mybir.AluOpType

    nc = bacc.Bacc("TRN2", target_bir_lowering=False, debug=False,
                   num_devices=N_CORES)

    x_d = nc.dram_tensor("x_sh", [BS, IN], F32R, kind="ExternalInput").ap()
    mu_d = nc.dram_tensor("mu_sh", [OS, IN], F32, kind="ExternalInput").ap()
    eps_d = nc.dram_tensor("eps_sh", [OS, IN], F32, kind="ExternalInput").ap()
    ls_d = nc.dram_tensor("ls_sh", [OS, IN], F32, kind="ExternalInput").ap()
    bmu_d = nc.dram_tensor("bmu_sh", [1, OS], F32, kind="ExternalInput").ap()
    bls_d = nc.dram_tensor("bls_sh", [1, OS], F32, kind="ExternalInput").ap()
    beps_d = nc.dram_tensor("beps_sh", [1, OS], F32, kind="ExternalInput").ap()
    o_d = nc.dram_tensor("o_sh", [BS, OS], F32, kind="ExternalOutput").ap()
    u_d = nc.dram_tensor("u_sh", [BS, OS], F32, kind="ExternalOutput").ap()

    with tile.TileContext(nc) as tc:
        with (
            tc.tile_pool(name="const", bufs=1) as cpool,
            tc.tile_pool(name="wres", bufs=1) as wres,
            tc.tile_pool(name="psum", bufs=3, space="PSUM") as ppool,
        ):
            ident_f = cpool.tile([128, 128], F32)
            make_identity(nc, ident_f)
            ident = cpool.tile([128, 128], F32R)
            nc.vector.tensor_copy(ident[:], ident_f[:])
            ones_f = cpool.tile([1, 128], F32)
            nc.vector.memset(ones_f[:], 1.0)
            ones1 = cpool.tile([1, 128], F32R)
            nc.vector.tensor_copy(ones1[:], ones_f[:])

            # --- weight prep: WsampT and S2T as KT k-tiles [128, OS] f32r
            wT = [wres.tile([128, OS], F32R, tag=f"wT{i}", name=f"wT{i}")
                  for i in range(KT)]
            s2T = [wres.tile([128, OS], F32R, tag=f"s2T{i}", name=f"s2T{i}")
                   for i in range(KT)]

            with (
                tc.tile_pool(name="wprep", bufs=2) as wpool,
                tc.tile_pool(name="xs", bufs=3) as xpool,
                tc.tile_pool(name="outs", bufs=3) as opool,
                tc.tile_pool(name="po", bufs=2, space="PSUM") as popool,
            ):
                state = {}

                HI = IN // 2

                def emit_jt(jt, h):
                    sl = slice(jt * 128, (jt + 1) * 128)
                    fsl = slice(h * HI, (h + 1) * HI)
                    mu_t = wpool.tile([128, HI], F32, tag="mu", name="mu_t",
                                      bufs=4)
                    eps_t = wpool.tile([128, HI], F32, tag="eps", name="eps_t",
                                       bufs=4)
                    nc.sync.dma_start(mu_t[:], mu_d[sl, fsl])
                    nc.sync.dma_start(eps_t[:], eps_d[sl, fsl])
                    w_t = wpool.tile([128, HI], F32R, tag="w", name="w_t",
                                     bufs=2)
                    ls_t = wpool.tile([128, HI], F32, tag="ls", name="ls_t",
                                      bufs=3)
                    nc.sync.dma_start(ls_t[:], ls_d[sl, fsl])
                    sig_t = wpool.tile([128, HI], F32, tag="sig",
                                       name="sig_t", bufs=2)
                    nc.scalar.activation(sig_t[:], ls_t[:], AF.Exp)
                    se_t = wpool.tile([128, HI], F32, tag="se", bufs=2,
                                      name="se_t")
                    nc.vector.tensor_tensor(se_t[:], sig_t[:], eps_t[:],
                                            ALU.mult)
                    nc.vector.tensor_tensor(w_t[:], mu_t[:], se_t[:], ALU.add)
                    s2_t = wpool.tile([128, HI], F32R, tag="s2", name="s2_t",
                                      bufs=2)
                    nc.scalar.activation(s2_t[:], sig_t[:], AF.Square)

                    k0 = h * (KT // 2)
                    for src_t, dst in ((w_t, wT), (s2_t, s2T)):
                        for g in range(KT // 8):
                            pt = ppool.tile([128, 512], F32R, tag="tp",
                                            name="pt")
                            for ii in range(4):
                                i = 4 * g + ii
                                nc.tensor.transpose(
                                    pt[:, ii * 128:(ii + 1) * 128],
                                    src_t[:, i * 128:(i + 1) * 128], ident[:])
                            for ii in range(4):
                                i = 4 * g + ii
                                nc.any.tensor_copy(
                                    dst[k0 + i][:, jt * 128:(jt + 1) * 128],
                                    pt[:, ii * 128:(ii + 1) * 128])

                def emit_front(bt):
                    x_t = xpool.tile([128, IN], F32R, tag="x", bufs=2,
                                     name="x_t")
                    dma_eng = nc.sync if bt % 2 == 0 else nc.scalar
                    dma_eng.dma_start(x_t[:], x_d[bt * 128:(bt + 1) * 128, :])
                    xT = xpool.tile([128, KT * 128], F32R, tag="xT", bufs=3,
                                    name="xT")
                    for g in range(KT // 4):
                        pt = ppool.tile([128, 512], F32R, tag="tp", name="pt")
                        for ii in range(4):
                            i = 4 * g + ii
                            nc.tensor.transpose(
                                pt[:, ii * 128:(ii + 1) * 128],
                                x_t[:, i * 128:(i + 1) * 128], ident[:])
                        nc.any.tensor_copy(xT[:, g * 512:(g + 1) * 512], pt[:])
                    state[bt] = xT

                def emit_back(bt):
                    xT = state.pop(bt)
                    po = popool.tile([128, OS], F32, tag="po", name="po")
                    for i in range(KT):
                        nc.tensor.matmul(po[:], xT[:, i * 128:(i + 1) * 128],
                                         wT[i][:], start=(i == 0),
                                         stop=(i == KT - 1))
                    o_t = opool.tile([128, OS], F32, tag="o", name="o_t",
                                     bufs=2)
                    nc.vector.tensor_tensor(o_t[:], po[:], bias_bc[:], ALU.add)
                    nc.sync.dma_start(o_d[bt * 128:(bt + 1) * 128, :], o_t[:])

                    u_t = opool.tile([128, OS], F32, tag="u", name="u_t",
                                     bufs=2)
                    x2T = xpool.tile([128, KT * 128], F32R, tag="x2T",
                                     bufs=1, name="x2T")
                    nc.scalar.activation(x2T[:], xT[:].bitcast(F32),
                                         AF.Square)
                    pu = popool.tile([128, OS], F32, tag="pu", name="pu",
                                     bufs=2)
                    for i in range(KT):
                        nc.tensor.matmul(pu[:],
                                         x2T[:, i * 128:(i + 1) * 128],
                                         s2T[i][:], start=(i == 0),
                                         stop=False)
                    nc.tensor.matmul(pu[:], ones1[:], bs2_r[:],
                                     start=False, stop=True)
                    nc.scalar.activation(u_t[:], pu[:], AF.Sqrt)
                    nc.sync.dma_start(u_d[bt * 128:(bt + 1) * 128, :], u_t[:])

                for jt in range(JT):
                    for h in range(2):
                        emit_jt(jt, h)

                # bias rows: b_samp = bmu + exp(bls)*beps ; bs2 = exp(2*bls)
                bmu_r = cpool.tile([1, OS], F32)
                bls_r = cpool.tile([1, OS], F32)
                beps_r = cpool.tile([1, OS], F32)
                nc.scalar.dma_start(bmu_r[:], bmu_d[:])
                nc.scalar.dma_start(bls_r[:], bls_d[:])
                nc.scalar.dma_start(beps_r[:], beps_d[:])
                bsig_r = cpool.tile([1, OS], F32)
                nc.scalar.activation(bsig_r[:], bls_r[:], AF.Exp)
                bse_r = cpool.tile([1, OS], F32)
                nc.vector.tensor_tensor(bse_r[:], bsig_r[:], beps_r[:],
                                        ALU.mult)
                bias_r = cpool.tile([1, OS], F32R)
                nc.vector.tensor_tensor(bias_r[:], bmu_r[:], bse_r[:], ALU.add)
                bs2_r = cpool.tile([1, OS], F32R)
                nc.vector.tensor_tensor(bs2_r[:], bsig_r[:], bsig_r[:],
                                        ALU.mult)

                # broadcast bias row across partitions (K=1 ones matmul)
                pb = ppool.tile([128, OS], F32, tag="tp")
                nc.tensor.matmul(pb[:], ones1[:], bias_r[:], start=True,
                                 stop=True)
                bias_bc = cpool.tile([128, OS], F32)
                nc.any.tensor_copy(bias_bc[:], pb[:])

                for bt in range(BT):
                    emit_front(bt)
                    emit_back(bt)

    nc.compile()
    return nc


# ---------------------------------------------------------------------------
# host wrapper
# ---------------------------------------------------------------------------

def kernel(x, weight_mu, weight_log_sigma, bias_mu, bias_log_sigma,
           eps_w, eps_b):
    global LAST_RESULT
    import ml_dtypes
    from concourse.bass_utils import run_bass_kernel_spmd

    BF = ml_dtypes.bfloat16

    x = np.ascontiguousarray(np.asarray(x, dtype=np.float32))
    weight_mu = np.asarray(weight_mu, dtype=np.float32)
    weight_log_sigma = np.asarray(weight_log_sigma, dtype=np.float32)
    bias_mu = np.asarray(bias_mu, dtype=np.float32).reshape(OUT)
    bias_log_sigma = np.asarray(bias_log_sigma, dtype=np.float32).reshape(OUT)
    eps_w = np.asarray(eps_w, dtype=np.float32)
    eps_b = np.asarray(eps_b, dtype=np.float32).reshape(OUT)

    ls0 = weight_log_sigma.flat[0]
    bls0 = bias_log_sigma.flat[0]
    fast = bool(np.all(weight_log_sigma == ls0)) and bool(
        np.all(bias_log_sigma == bls0))

    if fast:
        sigma = float(np.exp(np.float32(ls0)))
        sigma_b = float(np.exp(np.float32(bls0)))
        key = ("fast", sigma, sigma_b)
        if key not in _compiled:
            _compiled[key] = _build_fast(sigma, sigma_b)
        nc = _compiled[key]

        x_bf = x.astype(BF)
        xT_halves = [_pack_xT(x_bf[i * BS:(i + 1) * BS]) for i in range(R)]
        mu_bf = weight_mu.astype(BF)
        eps_bf = (eps_w * np.float32(sigma)).astype(BF)
        in_maps = []
        for i in range(R):
            for j in range(C):
                bv = np.zeros((8, 132), dtype=np.float32)
                bv[0:4, 0:128] = bias_mu[j * OS:(j + 1) * OS].reshape(4, 128)
                bv[4:8, 0:128] = eps_b[j * OS:(j + 1) * OS].reshape(4, 128)
                for n in range(4):
                    bv[n, 128 + n] = 1.0
                    bv[4 + n, 128 + n] = sigma_b
                in_maps.append({
                    "xT_sh": xT_halves[i],
                    "weps_sh": _pack_weps(mu_bf[j * OS:(j + 1) * OS],
                                          eps_bf[j * OS:(j + 1) * OS]),
                    "bv_sh": bv,
                })
        res = run_bass_kernel_spmd(nc, in_maps, core_ids=list(range(N_CORES)),
                                   trace=TRACE)
        LAST_RESULT = res

        output = np.empty((B, OUT), dtype=np.float32)
        uncertainty = np.empty((B, OUT), dtype=np.float32)
        for i in range(R):
            for j in range(C):
                c = i * C + j
                output[i * BS:(i + 1) * BS, j * OS:(j + 1) * OS] = (
                    res.results[c]["o_sh"].astype(np.float32).T)
            u_row = res.results[i * C]["u_sh"].reshape(BS)
            uncertainty[i * BS:(i + 1) * BS, :] = u_row[:, None]
        return output, uncertainty

    # ----- general fallback (original kernel) -----
    key = ("general",)
    if key not in _compiled:
        _compiled[key] = _build_general()
    nc = _compiled[key]

    bias_mu2 = bias_mu.reshape(1, OUT)
    bias_log_sigma2 = bias_log_sigma.reshape(1, OUT)
    eps_b2 = eps_b.reshape(1, OUT)
    in_maps = []
    for i in range(R):
        for j in range(C):
            m = {
                "x_sh": x[i * BS:(i + 1) * BS],
                "mu_sh": weight_mu[j * OS:(j + 1) * OS],
                "eps_sh": eps_w[j * OS:(j + 1) * OS],
                "ls_sh": weight_log_sigma[j * OS:(j + 1) * OS],
                "bmu_sh": bias_mu2[:, j * OS:(j + 1) * OS],
                "bls_sh": bias_log_sigma2[:, j * OS:(j + 1) * OS],
                "beps_sh": eps_b2[:, j * OS:(j + 1) * OS],
            }
            in_maps.append({k: np.ascontiguousarray(v) for k, v in m.items()})

    res = run_bass_kernel_spmd(nc, in_maps, core_ids=list(range(N_CORES)),
                               trace=TRACE)
    LAST_RESULT = res

    output = np.empty((B, OUT), dtype=np.float32)
    uncertainty = np.empty((B, OUT), dtype=np.float32)
    for i in range(R):
        for j in range(C):
            c = i * C + j
            output[i * BS:(i + 1) * BS,
                   j * OS:(j + 1) * OS] = res.results[c]["o_sh"]
            uncertainty[i * BS:(i + 1) * BS,
                        j * OS:(j + 1) * OS] = res.results[c]["u_sh"]
    return output, uncertainty
